# revision 8
# baseline (speedup 1.0000x reference)
"""Trainium2 Bass kernel for nn_MultiHeadSelfAttention_65429531788008.

Reference semantics (non-standard attention):
  q,k,v = x@W* + b*          [B,T,H,64]
  scores[b,h,tk,tq] = q[b,tq,h]·k[b,tk,h]
  attn = softmax(scores/8, axis=tq)         (softmax over QUERY axis, per tk row)
  colsum[b,h,tq] = sum_tk attn[b,h,tk,tq]
  out = (v * colsum[...,None]).reshape(B,T,1024) @ Wo + bo

Sharding: 8 cores = 2 batches x 4 head-groups (4 heads each). Each core
computes its batch/head-group partial output [T,1024] = (v_loc*colsum)@Wo_rows;
host sums the 4 partials per batch and adds bo.
"""
import os
import sys
import time
from contextlib import ExitStack

import numpy as np

sys.path.insert(0, "/opt/trn_rl_repo")

import concourse.bass as bass  # noqa: E402
import concourse.tile as tile  # noqa: E402
from concourse import bacc, mybir  # noqa: E402

N_CORES = 8
B, T, DM = 2, 2048, 1024
H, D = 16, 64
HPC = H // (N_CORES // B)   # heads per core = 4
PAIRS = HPC // 2            # head pairs per core = 2
HD = HPC * D                # 256 local head dims
F32 = mybir.dt.float32
F32R = mybir.dt.float32r
AF = mybir.ActivationFunctionType


def build(T=T, DM=DM, HD=HD, n_cores=N_CORES):
    """Build the SPMD Bacc program (identical on all cores)."""
    PAIRS = HD // 128           # head pairs
    NB_DM = DM // 128           # dm contraction blocks
    TKB = T // 128              # tk blocks per head
    NCH = T // 512              # 512-wide tq chunks
    NHF = T // 1024             # 1024-wide tq halves

    nc = bacc.Bacc("TRN2", target_bir_lowering=False, debug=False,
                   num_devices=n_cores)
    xT = nc.dram_tensor("xT", [DM, T], F32, kind="ExternalInput").ap()
    wq = nc.dram_tensor("wq", [DM, HD], F32, kind="ExternalInput").ap()
    wk = nc.dram_tensor("wk", [DM, HD], F32, kind="ExternalInput").ap()
    wv = nc.dram_tensor("wv", [DM, HD], F32, kind="ExternalInput").ap()
    bq = nc.dram_tensor("bq", [HD, 1], F32, kind="ExternalInput").ap()
    bk = nc.dram_tensor("bk", [HD, 1], F32, kind="ExternalInput").ap()
    bv = nc.dram_tensor("bv", [HD, 1], F32, kind="ExternalInput").ap()
    wo = nc.dram_tensor("wo", [HD, DM], F32, kind="ExternalInput").ap()
    mask = nc.dram_tensor("mask", [2, 128], F32, kind="ExternalInput").ap()
    out = nc.dram_tensor("out", [T, DM], F32, kind="ExternalOutput").ap()

    with tile.TileContext(nc) as tc, ExitStack() as ctx:
        # ---- pools that live for the whole kernel ----
        qkv = ctx.enter_context(tc.tile_pool(name="qkv", bufs=1))
        consts = ctx.enter_context(tc.tile_pool(name="consts", bufs=1))
        cs_sb = ctx.enter_context(tc.tile_pool(name="cs_sb", bufs=1))

        q_t = [qkv.tile([128, T], F32R, tag=f"q{p}", name=f"q{p}") for p in range(PAIRS)]
        k_t = [qkv.tile([128, T], F32R, tag=f"k{p}", name=f"k{p}") for p in range(PAIRS)]
        v_t = [qkv.tile([128, T], F32R, tag=f"v{p}", name=f"v{p}") for p in range(PAIRS)]

        mask_t = consts.tile([2, 128], F32R)
        nc.gpsimd.dma_start(out=mask_t, in_=mask)
        wo_t = [consts.tile([128, DM], F32R, tag=f"wo{p}", name=f"wo{p}") for p in range(PAIRS)]
        for p in range(PAIRS):
            nc.gpsimd.dma_start(out=wo_t[p], in_=wo[p * 128:(p + 1) * 128, :])
        bias_t = {}
        for nm, bap in (("q", bq), ("k", bk), ("v", bv)):
            for p in range(PAIRS):
                bt = consts.tile([128, 1], F32, tag=f"b{nm}{p}", name=f"b{nm}{p}")
                nc.sync.dma_start(out=bt, in_=bap[p * 128:(p + 1) * 128, :])
                bias_t[(nm, p)] = bt
        # colsum staging [1, NCH, 512] per (pair, head)
        colsum_sb = [[cs_sb.tile([1, NCH, 512], F32R, tag=f"cs{p}{h}",
                                 name=f"cs{p}{h}") for h in range(2)]
                     for p in range(PAIRS)]

        # ================= Phase 1: projections =================
        with ExitStack() as p1:
            xt_pool = p1.enter_context(tc.tile_pool(name="xt", bufs=1))
            wt_pool = p1.enter_context(tc.tile_pool(name="wt", bufs=1))
            p1ps = p1.enter_context(tc.tile_pool(name="p1ps", bufs=2, space="PSUM"))

            xt_t = []
            for d in range(NB_DM):
                xt = xt_pool.tile([128, T], F32R, tag=f"xt{d}", name=f"xt{d}")
                nc.gpsimd.dma_start(out=xt, in_=xT[d * 128:(d + 1) * 128, :])
                xt_t.append(xt)
            w_t = {}
            for nm, wap in (("k", wk), ("q", wq), ("v", wv)):
                for d in range(NB_DM):
                    wt = wt_pool.tile([128, HD], F32R, tag=f"w{nm}{d}", name=f"w{nm}{d}")
                    nc.gpsimd.dma_start(out=wt, in_=wap[d * 128:(d + 1) * 128, :])
                    w_t[(nm, d)] = wt

            # K first, then Q (phase 2 pair-0 can start earliest), then V
            for nm, dest in (("k", k_t), ("q", q_t), ("v", v_t)):
                for p in range(PAIRS):
                    ps_g = p1ps.tile([128, T], F32, tag="p1ps", name="p1psg")
                    for d in range(NB_DM):
                        lhsT = w_t[(nm, d)][:, p * 128:(p + 1) * 128]
                        for c in range(NCH):
                            nc.tensor.matmul(
                                ps_g[:, c * 512:(c + 1) * 512], lhsT,
                                xt_t[d][:, c * 512:(c + 1) * 512],
                                start=(d == 0), stop=(d == NB_DM - 1))
                    # PSUM -> SBUF with per-partition bias add (rounds to f32r)
                    nc.scalar.activation(dest[p][:], ps_g[:], AF.Identity,
                                         bias=bias_t[(nm, p)][:], scale=1.0)

        # ================= Phase 2: scores/softmax/colsum =================
        with ExitStack() as p2:
            sc_ps = p2.enter_context(tc.tile_pool(name="sc_ps", bufs=2, space="PSUM"))
            cs_ps = p2.enter_context(tc.tile_pool(name="cs_ps", bufs=4, space="PSUM"))
            ep = p2.enter_context(tc.tile_pool(name="exp", bufs=6))
            sp = p2.enter_context(tc.tile_pool(name="small", bufs=16))

            for p in range(PAIRS):
                for h in range(2):
                    hb = h * 64
                    csp = [cs_ps.tile([1, 512], F32, tag="cs_ps", name="csps")
                           for _ in range(NCH)]
                    for blk in range(TKB):
                        exp_t = {}
                        racc = {}
                        for half in range(NHF):
                            ps_t = sc_ps.tile([128, 1024], F32, tag="sc",
                                              name="scps")
                            for c2 in range(2):
                                cix = half * 2 + c2
                                nc.tensor.matmul(
                                    ps_t[:, c2 * 512:(c2 + 1) * 512],
                                    k_t[p][hb:hb + 64, blk * 128:(blk + 1) * 128],
                                    q_t[p][hb:hb + 64, cix * 512:(cix + 1) * 512],
                                    start=True, stop=True)
                            et = ep.tile([128, 1024], F32R, tag="exp", name="expt")
                            ra = sp.tile([128, 1], F32, tag="racc", name="racc")
                            nc.scalar.activation(et[:], ps_t[:], AF.Exp,
                                                 bias=0.0, scale=0.125,
                                                 accum_out=ra[:])
                            exp_t[half] = et
                            racc[half] = ra
                        if NHF == 1:
                            s_t = racc[0]
                        else:
                            s_t = sp.tile([128, 1], F32, tag="s", name="s")
                            nc.vector.tensor_add(s_t[:], racc[0][:], racc[1][:])
                        ci = sp.tile([128, 1], F32, tag="ci", name="ci")
                        nc.vector.reciprocal(ci[:], s_t[:])
                        cr = sp.tile([128, 1], F32R, tag="cr", name="cr")
                        nc.vector.tensor_copy(cr[:], ci[:])
                        for half in range(NHF):
                            for c2 in range(2):
                                cix = half * 2 + c2
                                nc.tensor.matmul(
                                    csp[cix][:], cr[:],
                                    exp_t[half][:, c2 * 512:(c2 + 1) * 512],
                                    start=(blk == 0), stop=(blk == TKB - 1))
                    # evacuate colsum accumulators -> SBUF (f32r)
                    for cix in range(NCH):
                        nc.vector.tensor_copy(
                            colsum_sb[p][h][0:1, cix, :], csp[cix][:])

        # ================= Phase 3: mixed + output projection =================
        with ExitStack() as p3:
            p3ps = p3.enter_context(tc.tile_pool(name="p3ps", bufs=4, space="PSUM"))
            mx = p3.enter_context(tc.tile_pool(name="mx", bufs=1))
            ost = p3.enter_context(tc.tile_pool(name="ost", bufs=3))

            mixed_t = [mx.tile([128, T], F32R, tag=f"mx{p}", name=f"mx{p}") for p in range(PAIRS)]
            for p in range(PAIRS):
                # stack both heads' colsum rows onto partitions 0/1 via DMA
                cs2 = mx.tile([2, NCH, 512], F32R, tag=f"cs2_{p}", name=f"cs2_{p}")
                for h in range(2):
                    nc.sync.dma_start(out=cs2[h:h + 1, :, :],
                                      in_=colsum_sb[p][h][0:1, :, :])
                for cix in range(NCH):
                    bc = p3ps.tile([128, 512], F32, tag="bc", name="bcps")
                    nc.tensor.matmul(bc[:], mask_t[:], cs2[:, cix, :],
                                     start=True, stop=True)
                    nc.vector.tensor_mul(
                        mixed_t[p][:, cix * 512:(cix + 1) * 512],
                        v_t[p][:, cix * 512:(cix + 1) * 512], bc[:])
            for blk in range(T // 128):
                stg = ost.tile([128, DM], F32, tag="ost", name="ostg")
                for m in range(DM // 512):
                    po = p3ps.tile([128, 512], F32, tag="po", name="pops")
                    for p in range(PAIRS):
                        nc.tensor.matmul(
                            po[:], mixed_t[p][:, blk * 128:(blk + 1) * 128],
                            wo_t[p][:, m * 512:(m + 1) * 512],
                            start=(p == 0), stop=(p == PAIRS - 1))
                    nc.vector.tensor_copy(stg[:, m * 512:(m + 1) * 512], po[:])
                nc.sync.dma_start(out=out[blk * 128:(blk + 1) * 128, :], in_=stg[:])

    nc.compile()
    return nc


_MASK = np.zeros((2, 128), np.float32)
_MASK[0, :64] = 1.0
_MASK[1, 64:] = 1.0


def make_in_maps(x, Wq, bq, Wk, bk, Wv, bv, Wo):
    """Shard full inputs into per-core in_maps (host side)."""
    in_maps = []
    gpc = H // (N_CORES // B)  # heads per core
    for c in range(N_CORES):
        b = c // (N_CORES // B)
        hg = c % (N_CORES // B)
        sl = slice(hg * gpc * D, (hg + 1) * gpc * D)
        in_maps.append({
            "xT": np.ascontiguousarray(x[b].T),
            "wq": np.ascontiguousarray(Wq[:, sl]),
            "wk": np.ascontiguousarray(Wk[:, sl]),
            "wv": np.ascontiguousarray(Wv[:, sl]),
            "bq": np.ascontiguousarray(bq[sl].reshape(-1, 1)),
            "bk": np.ascontiguousarray(bk[sl].reshape(-1, 1)),
            "bv": np.ascontiguousarray(bv[sl].reshape(-1, 1)),
            "wo": np.ascontiguousarray(Wo[sl, :]),
            "mask": _MASK,
        })
    return in_maps


def gather(results, bo):
    """Sum per-core partials into the full [B,T,DM] output, add bo."""
    out = np.zeros((B, T, DM), np.float32)
    cpb = N_CORES // B
    for c in range(N_CORES):
        out[c // cpb] += results[c]["out"]
    return (out + bo.reshape(1, 1, -1)).astype(np.float32)


_NC = None


def _get_nc():
    global _NC
    if _NC is None:
        _NC = build()
    return _NC


def kernel(x, Wq, bq, Wk, bk, Wv, bv, Wo, bo):
    from concourse.bass_utils import run_bass_kernel_spmd
    x = np.asarray(x, np.float32)
    in_maps = make_in_maps(x, np.asarray(Wq), np.asarray(bq), np.asarray(Wk),
                           np.asarray(bk), np.asarray(Wv), np.asarray(bv),
                           np.asarray(Wo))
    nc = _get_nc()
    res = run_bass_kernel_spmd(nc, in_maps, core_ids=list(range(N_CORES)))
    return gather(res.results, np.asarray(bo))
